# revision 5
# baseline (speedup 1.0000x reference)
"""BinaryLinear kernel for Trainium2 (8 NeuronCores, SPMD).

Computes y = x @ sign(W)^T + sign(b) with x:[8192,4096] f32,
W:[4096,4096] f32, b:[4096] f32.

Sharding: 2-way over tokens x 4-way over out_features (8 cores).
Per core: x_shard [4096, 4096], W_shard [1024, 4096], b_shard [1024]
-> y_shard [4096, 1024]. No collectives; host shards/concats.

Math strategy: sign(W) is exactly representable in bf16 (+-1), and a
SINGLE bf16 pass (y = bf16(x) @ sW^T accumulated in f32 PSUM) gives
~1.7e-3 max-metric relative error -- well under the 2e-2 tolerance.
PE work per core: 2048 LDW+MM pairs (N=512) ~ 437 us.

v2 changes vs v1 (v1 = 702us HW, first MM at 119us, DMA union 524us):
  - x and W are cast-loaded f32->bf16 during the SWDGE DMA itself
    (gpsimd dtype-cast DMA). Removes the per-tile DVE cast from the
    x critical chain and halves SBUF-port write traffic for x/W.
  - Phase 0 restructured: ALL cast-loads issue first on the SWDGE
    ring (x0, W0-3, x1, W4-7), signs on DVE (bit trick, in-place),
    then the transposes run on the SP ring in order
    [x0T, W0T..W3T, x1T, W4T..W7T]. Matmuls for tile 0's first out
    group need only x0T+W0-3T, so the PE starts while W4-7 still
    transpose (og-progressive via emission order + Tile deps).
  - Phase 1 prefetch depth 2 (x cast-load for tt+2 issued before
    mm_tile(tt)).

Hardware constraints baked into this structure (learned from NTFF
traces and device crashes):
  - A DMA transpose occupies all 16 DMA engines: it is mutually
    exclusive with copy DMAs and pays a ~10us drain when copies are in
    flight. Keep the SP queue transposes-only; batch transposes after
    the bulk loads complete.
  - Concurrent transposes issued from two HWDGE queues, or matmuls
    racing a transpose into the same SBUF tile, crash the device
    (NRT_EXEC_UNIT_UNRECOVERABLE).
"""

import sys

sys.path.insert(0, "/opt/trn_rl_repo")

import numpy as np

import concourse.bass as bass  # noqa: F401
import concourse.mybir as mybir
from concourse import bacc, tile
from concourse.bass_utils import run_bass_kernel_spmd

TOKENS, IN, OUT = 8192, 4096, 4096
N_CORES = 8
T_SPLIT, O_SPLIT = 2, 4
T_CORE, O_CORE = TOKENS // T_SPLIT, OUT // O_SPLIT

P = 128
FREE = 512  # matmul moving free dim / psum bank width (f32)

F32 = mybir.dt.float32
BF16 = mybir.dt.bfloat16
U16 = mybir.dt.uint16


def emit(nc, tc, x_d, w_d, b_d, y_d, t_core, in_dim, o_core):
    """Emit the per-core program. x_d [t_core, in], w_d [o_core, in],
    b_d [1, o_core], y_d [t_core, o_core]."""
    KS = in_dim // P  # number of 128-wide k slabs
    TT = t_core // P  # token tiles
    OT = o_core // P  # 128-row tiles of W

    from contextlib import ExitStack

    def sign_bits_inplace(t):
        # (bits & 0x8000) | 0x3F80 == +-1.0 bf16, applied in place (DVE)
        nc.vector.tensor_scalar(
            out=t.bitcast(U16),
            in0=t.bitcast(U16),
            scalar1=0x8000,
            scalar2=0x3F80,
            op0=mybir.AluOpType.bitwise_and,
            op1=mybir.AluOpType.bitwise_or,
        )

    with ExitStack() as ctx:
        const = ctx.enter_context(tc.tile_pool(name="const", bufs=1))
        # Resident sign(W)^T: [128 k-part, KS slabs, o_core] bf16
        swt = const.tile([P, KS, o_core], BF16)
        bias_bc = const.tile([P, o_core], F32)

        with (
            tc.tile_pool(name="wload", bufs=8) as wpool,
            tc.tile_pool(name="xin", bufs=3) as hpool,
            tc.tile_pool(name="xt", bufs=3) as tpool,
            tc.tile_pool(name="psum", bufs=8, space="PSUM") as psum,
            tc.tile_pool(name="yout", bufs=3) as opool,
        ):

            def load_x(tt):
                """x f32 -> bf16 cast-load on the SWDGE ring."""
                trow = slice(tt * P, (tt + 1) * P)
                xh = hpool.tile([P, in_dim], BF16, name="xh")
                nc.gpsimd.dma_start(xh, x_d[trow, :])
                return xh

            def transpose_x(xh):
                xT = tpool.tile([P, KS, P], BF16, name="xT")
                nc.sync.dma_start_transpose(xT, xh)
                return xT

            # ---- Phase 0: cast-loads, signs, transposes ----
            braw = wpool.tile([P, o_core], F32, name="braw", bufs=1)
            nc.gpsimd.dma_start(braw, b_d.to_broadcast([P, o_core]))
            nc.scalar.sign(bias_bc, braw)

            xh0 = load_x(0)
            wfs = []
            for ot in range(OT):
                if ot == 4:
                    xh1 = load_x(1)
                wf = wpool.tile([P, in_dim], BF16, name="wf")
                nc.gpsimd.dma_start(wf, w_d[ot * P : (ot + 1) * P, :])
                wfs.append(wf)
            for wf in wfs:
                sign_bits_inplace(wf)

            # Transposes on the SP ring, in FIFO order:
            xT0 = transpose_x(xh0)
            for ot in range(4):
                nc.sync.dma_start_transpose(
                    swt[:, :, ot * P : (ot + 1) * P], wfs[ot]
                )
            xT1 = transpose_x(xh1)
            for ot in range(4, OT):
                nc.sync.dma_start_transpose(
                    swt[:, :, ot * P : (ot + 1) * P], wfs[ot]
                )

            # ---- Phase 1 ----
            def sweep(ps, xT, ocol, width):
                for ks in range(KS):
                    nc.tensor.matmul(
                        ps[:, :width], xT[:, ks, :], swt[:, ks, ocol],
                        start=(ks == 0), stop=(ks == KS - 1),
                    )

            def mm_tile(tt, xT):
                """matmul sweeps in FREE-wide out groups + bias evict."""
                trow = slice(tt * P, (tt + 1) * P)
                yo = opool.tile([P, o_core], F32, name="yo")
                for og in range(o_core // FREE):
                    ocol = slice(og * FREE, (og + 1) * FREE)
                    ps = psum.tile([P, FREE], F32, name="ps")
                    sweep(ps, xT, ocol, FREE)
                    nc.vector.tensor_tensor(
                        out=yo[:, ocol], in0=ps[:, :FREE],
                        in1=bias_bc[:, ocol], op=mybir.AluOpType.add,
                    )
                nc.gpsimd.dma_start(y_d[trow, :], yo)

            pend = {0: xT0, 1: xT1}
            for tt in range(TT):
                if tt + 2 < TT:
                    pend[tt + 2] = transpose_x(load_x(tt + 2))
                mm_tile(tt, pend.pop(tt))


def build(t_core=T_CORE, in_dim=IN, o_core=O_CORE):
    nc = bacc.Bacc("TRN2", target_bir_lowering=False, debug=False)
    x_d = nc.dram_tensor("x", [t_core, in_dim], F32, kind="ExternalInput")
    w_d = nc.dram_tensor("w", [o_core, in_dim], F32, kind="ExternalInput")
    b_d = nc.dram_tensor("b", [1, o_core], F32, kind="ExternalInput")
    y_d = nc.dram_tensor("y", [t_core, o_core], F32, kind="ExternalOutput")
    with tile.TileContext(nc) as tc:
        emit(nc, tc, x_d.ap(), w_d.ap(), b_d.ap(), y_d.ap(), t_core, in_dim, o_core)
    nc.compile()
    return nc


_nc_cache = None


def kernel(x: np.ndarray, weight: np.ndarray, bias: np.ndarray, **run_kwargs):
    global _nc_cache
    if _nc_cache is None:
        _nc_cache = build()
    nc = _nc_cache

    x = np.ascontiguousarray(x, dtype=np.float32)
    weight = np.ascontiguousarray(weight, dtype=np.float32)
    bias = np.ascontiguousarray(bias, dtype=np.float32)

    in_maps = []
    for c in range(N_CORES):
        th, oq = divmod(c, O_SPLIT)
        in_maps.append(
            {
                "x": x[th * T_CORE : (th + 1) * T_CORE],
                "w": weight[oq * O_CORE : (oq + 1) * O_CORE],
                "b": bias[oq * O_CORE : (oq + 1) * O_CORE].reshape(1, O_CORE),
            }
        )
    res = run_bass_kernel_spmd(nc, in_maps, core_ids=list(range(N_CORES)), **run_kwargs)
    y = np.empty((TOKENS, OUT), dtype=np.float32)
    for c in range(N_CORES):
        th, oq = divmod(c, O_SPLIT)
        y[th * T_CORE : (th + 1) * T_CORE, oq * O_CORE : (oq + 1) * O_CORE] = (
            res.results[c]["y"]
        )
    kernel.last_results = res
    return y


# revision 7
# speedup vs baseline: 1.0509x; 1.0509x over previous
"""BinaryLinear kernel for Trainium2 (8 NeuronCores, SPMD).

Computes y = x @ sign(W)^T + sign(b) with x:[8192,4096] f32,
W:[4096,4096] f32, b:[4096] f32.

Sharding: 2-way over tokens x 4-way over out_features (8 cores).
Per core: x_shard [4096, 4096], W_shard [1024, 4096], b_shard [1024]
-> y_shard [4096, 1024]. No collectives; host shards/concats.

Math strategy: sign(W) is exactly representable in bf16 (+-1), and a
SINGLE bf16 pass (y = bf16(x) @ sW^T accumulated in f32 PSUM) gives
~1.7e-3 max-metric relative error -- well under the 2e-2 tolerance.
PE work per core: 2048 LDW+MM pairs (N=512) ~ 437 us.

v3 notes (v1=702us: first MM 119us, ~3.2us/tile of gaps; v2=777us:
SWDGE f32->bf16 cast-loads run at ~150GB/s and starve the SP ring --
reverted):
  - x processed in PAIRS of 128-token tiles: one f32 pair-load
    [128, 2, 4096] (4MB), one DVE cast to bf16 [128, 8192], ONE xbar
    transpose per pair -> [128, 64, 128] (slabs 0-31 = even tile's k,
    32-63 = odd tile's k). Halves transpose count/fixed costs.
  - Phase 0 in two load->transpose windows: [W0-3 + x pair0 loads]
    [T: x01, W0-3] [W4-7 loads, og0 matmuls for tiles 0-1 run here]
    [T: W4-7] [og1 matmuls + steady pipeline]. First MM needs only
    x01T + W0-3T (og-progressive via emission order + Tile deps).
  - W signs via DVE bit-trick reading the f32 high halfwords
    ((hi16 & 0x8000) | 0x3F80 == +-1.0 bf16), W loads on the scalar
    HWDGE ring (wf ring of 1 serializes load ot+1 behind sign ot,
    which keeps SBUF fit: ~206KB/partition peak).
  - bias kept as bf16 (eviction adds bf16 bias to f32 PSUM on DVE).

Hardware constraints baked into this structure (learned from NTFF
traces and device crashes):
  - A DMA transpose occupies all 16 DMA engines: it is mutually
    exclusive with copy DMAs and pays a drain penalty when copies are
    in flight. Keep the SP queue transposes-only; batch transposes
    after the bulk loads of each window complete.
  - Concurrent transposes issued from two HWDGE queues, or matmuls
    racing a transpose into the same SBUF tile, crash the device
    (NRT_EXEC_UNIT_UNRECOVERABLE).
"""

import sys

sys.path.insert(0, "/opt/trn_rl_repo")

import numpy as np

import concourse.bass as bass  # noqa: F401
import concourse.mybir as mybir
from concourse import bacc, tile
from concourse.bass_utils import run_bass_kernel_spmd

TOKENS, IN, OUT = 8192, 4096, 4096
N_CORES = 8
T_SPLIT, O_SPLIT = 2, 4
T_CORE, O_CORE = TOKENS // T_SPLIT, OUT // O_SPLIT

P = 128
FREE = 512  # matmul moving free dim / psum bank width (f32)

F32 = mybir.dt.float32
BF16 = mybir.dt.bfloat16
U16 = mybir.dt.uint16


def emit(nc, tc, x_d, w_d, b_d, y_d, t_core, in_dim, o_core):
    """Emit the per-core program. x_d [t_core, in], w_d [o_core, in],
    b_d [1, o_core], y_d [t_core, o_core]."""
    KS = in_dim // P  # number of 128-wide k slabs
    TT = t_core // P  # token tiles
    NP = TT // 2  # token tile pairs
    OT = o_core // P  # 128-row tiles of W
    OG = o_core // FREE

    from contextlib import ExitStack

    with ExitStack() as ctx:
        const = ctx.enter_context(tc.tile_pool(name="const", bufs=1))
        # Resident sign(W)^T: [128 k-part, KS slabs, o_core] bf16
        swt = const.tile([P, KS, o_core], BF16)
        bias_bc = const.tile([P, o_core], BF16)

        xfp = ctx.enter_context(tc.tile_pool(name="xf", bufs=1))
        hp = ctx.enter_context(tc.tile_pool(name="xh", bufs=1))
        tp = ctx.enter_context(tc.tile_pool(name="xt", bufs=2))
        psum = ctx.enter_context(tc.tile_pool(name="psum", bufs=8, space="PSUM"))
        op = ctx.enter_context(tc.tile_pool(name="yout", bufs=2))

        def load_pair(p):
            """f32 pair-load: tokens [256p, 256p+256) as [128, 2, 4096]."""
            xf = xfp.tile([P, 2, in_dim], F32, name="xf")
            src = x_d[256 * p : 256 * (p + 1), :].rearrange(
                "(i p) k -> p i k", p=P
            )
            nc.gpsimd.dma_start(xf, src)
            return xf

        def cast_pair(xf):
            xh = hp.tile([P, 2 * in_dim], BF16, name="xh")
            nc.vector.tensor_copy(
                out=xh, in_=xf.rearrange("p i k -> p (i k)")
            )
            return xh

        def transpose_pair(xh):
            xT = tp.tile([P, 2 * KS, P], BF16, name="xT")
            nc.sync.dma_start_transpose(xT, xh)
            return xT

        def sweep(tt, xT, og):
            """One 512-wide out-group accumulation + bias eviction for
            token tile tt whose k-slabs live in xT at offset 32*(tt%2)."""
            base = KS * (tt % 2)
            ocol = slice(og * FREE, (og + 1) * FREE)
            ps = psum.tile([P, FREE], F32, name="ps")
            for ks in range(KS):
                nc.tensor.matmul(
                    ps, xT[:, base + ks, :], swt[:, ks, ocol],
                    start=(ks == 0), stop=(ks == KS - 1),
                )
            yo = op.tile([P, FREE], F32, name="yo")
            nc.vector.tensor_tensor(
                out=yo, in0=ps, in1=bias_bc[:, ocol],
                op=mybir.AluOpType.add,
            )
            trow = slice(tt * P, (tt + 1) * P)
            nc.gpsimd.dma_start(y_d[trow, ocol], yo)

        with tc.tile_pool(name="wload", bufs=1) as wpool:
            # bias: broadcast-load + sign (ACT, f32 -> bf16)
            braw = wpool.tile([P, o_core], F32, name="braw", bufs=1)
            nc.gpsimd.dma_start(braw, b_d.to_broadcast([P, o_core]))
            nc.scalar.sign(bias_bc, braw)

            def w_load_sign(ot):
                wf = wpool.tile([P, in_dim], F32, name="wf", bufs=1)
                nc.scalar.dma_start(wf, w_d[ot * P : (ot + 1) * P, :])
                ws = wpool.tile([P, in_dim], BF16, name="ws", bufs=4)
                nc.vector.tensor_scalar(
                    out=ws.bitcast(U16),
                    in0=wf.bitcast(U16)[:, 1::2],
                    scalar1=0x8000,
                    scalar2=0x3F80,
                    op0=mybir.AluOpType.bitwise_and,
                    op1=mybir.AluOpType.bitwise_or,
                )
                return ws

            def w_transpose(ot, ws):
                nc.sync.dma_start_transpose(
                    swt[:, :, ot * P : (ot + 1) * P], ws
                )

            # ---- Phase 0 ----
            # window 1: x pair0 + W0-3 loads; then transposes [x01, W0-3]
            xf0 = load_pair(0)
            wss = [w_load_sign(ot) for ot in range(4)]
            xh0 = cast_pair(xf0)
            xT0 = transpose_pair(xh0)
            for ot in range(4):
                w_transpose(ot, wss[ot])

            # window 2: W4-7 loads + x pair1; og0 matmuls for tiles 0-1
            xf1 = load_pair(1)
            wss2 = [w_load_sign(ot) for ot in range(4, OT)]
            sweep(0, xT0, 0)
            sweep(1, xT0, 0)
            xh1 = cast_pair(xf1)
            xT1 = transpose_pair(xh1)
            for ot in range(4, OT):
                w_transpose(ot, wss2[ot - 4])

            # og1 catch-up for tiles 0-1
            sweep(0, xT0, 1)
            sweep(1, xT0, 1)

        # ---- Phase 1 (steady pairs) ----
        pend = {1: xT1}
        for p in range(1, NP):
            if p + 1 < NP:
                pend[p + 1] = transpose_pair(cast_pair(load_pair(p + 1)))
            xT = pend.pop(p)
            for tt in (2 * p, 2 * p + 1):
                for og in range(OG):
                    sweep(tt, xT, og)


def build(t_core=T_CORE, in_dim=IN, o_core=O_CORE):
    nc = bacc.Bacc("TRN2", target_bir_lowering=False, debug=False)
    x_d = nc.dram_tensor("x", [t_core, in_dim], F32, kind="ExternalInput")
    w_d = nc.dram_tensor("w", [o_core, in_dim], F32, kind="ExternalInput")
    b_d = nc.dram_tensor("b", [1, o_core], F32, kind="ExternalInput")
    y_d = nc.dram_tensor("y", [t_core, o_core], F32, kind="ExternalOutput")
    with tile.TileContext(nc) as tc:
        emit(nc, tc, x_d.ap(), w_d.ap(), b_d.ap(), y_d.ap(), t_core, in_dim, o_core)
    nc.compile()
    return nc


_nc_cache = None


def kernel(x: np.ndarray, weight: np.ndarray, bias: np.ndarray, **run_kwargs):
    global _nc_cache
    if _nc_cache is None:
        _nc_cache = build()
    nc = _nc_cache

    x = np.ascontiguousarray(x, dtype=np.float32)
    weight = np.ascontiguousarray(weight, dtype=np.float32)
    bias = np.ascontiguousarray(bias, dtype=np.float32)

    in_maps = []
    for c in range(N_CORES):
        th, oq = divmod(c, O_SPLIT)
        in_maps.append(
            {
                "x": x[th * T_CORE : (th + 1) * T_CORE],
                "w": weight[oq * O_CORE : (oq + 1) * O_CORE],
                "b": bias[oq * O_CORE : (oq + 1) * O_CORE].reshape(1, O_CORE),
            }
        )
    res = run_bass_kernel_spmd(nc, in_maps, core_ids=list(range(N_CORES)), **run_kwargs)
    y = np.empty((TOKENS, OUT), dtype=np.float32)
    for c in range(N_CORES):
        th, oq = divmod(c, O_SPLIT)
        y[th * T_CORE : (th + 1) * T_CORE, oq * O_CORE : (oq + 1) * O_CORE] = (
            res.results[c]["y"]
        )
    kernel.last_results = res
    return y
